# revision 10
# baseline (speedup 1.0000x reference)
"""Trainium2 Bass kernel for the YOLO-style DetectionLayer.

Reference computation (per batch b, anchor a, grid cell (gy, gx)):
    pred = x[b].reshape(3, 85, 76, 76)  channels-first per anchor
    bx = (sigmoid(tx) + gx) * stride        stride = 608/76 = 8
    by = (sigmoid(ty) + gy) * stride
    bw = exp(tw) * anchor_w                 (stride cancels)
    bh = exp(th) * anchor_h
    conf/cls = sigmoid(...)
    out[b, a*5776 + gy*76 + gx, :] = [bx, by, bw, bh, conf, cls0..79]

Strategy (pure data-parallel over batch, 8 cores x 4 images):
  * Per (b, a) slab: DMA [85 ch, 5776 px] f32 -> SBUF (channels on
    partitions).
  * One ACT pass: sigmoid over all 85 rows, rounding to bf16 (the
    harness tolerates rel err < 2e-2; bf16 keeps us ~3 decimal digits).
    exp is derived on DVE as s/(1-s) to avoid the ~2.7us ACT table
    switch between the sigmoid and exp sets.
  * TensorE transpose-mode matmuls (bf16, 2x fp32 rate) flip
    [85, 128px] -> PSUM [128px, 85ch].  Pixels are interleaved stride-6
    so each SBUF output partition holds 6 consecutive output rows =
    1020 contiguous bytes in DRAM per partition (>=512B keeps the DMA
    at full rate).
  * Box fix-ups run in the transposed layout where box channels are a
    few free-dim columns across all 128 partitions (3-4 DVE ops/slab).
  * One big bf16 store DMA per slab -- HALF the bytes of an f32 store.
    The host widens bf16 -> f32 at gather time (exact bit-shift).
"""

from contextlib import ExitStack

import ml_dtypes
import numpy as np

import concourse.bacc as bacc
import concourse.mybir as mybir
import concourse.tile as tile
from concourse.bass_utils import run_bass_kernel_spmd

F32 = mybir.dt.float32
BF16 = mybir.dt.bfloat16
Alu = mybir.AluOpType
Act = mybir.ActivationFunctionType

N_CORES = 8
NA = 3  # anchors
NCH = 85  # 5 + 80 classes
G = 76
GG = G * G  # 5776
STRIDE = 8.0

# pixel chunking for the transpose: 7 chunks of 128 partitions x 6 px
# (stride-6 interleave), tail chunk of 100 partitions x 4 px.
NJ, KI, KK = 7, 128, 6  # main: 7 * 768 px
TI, TK = 100, 4  # tail: 400 px
MAIN_PX = NJ * KI * KK  # 5376
MAIN_COLS = KK * NCH  # 510
TAIL_COLS = TK * NCH  # 340
OUT_COLS = NJ * MAIN_COLS + TAIL_COLS  # 3910

# grid8 / inva column layout: main j<7: q = j*12 + kk*2 + c ; tail: 84 + kk*2 + c
QCOLS = NJ * KK * 2 + TK * 2  # 92


def _build(
    nb: int,
    inp_bufs: int = 2,
    sig_bufs: int = 2,
    out_bufs: int = 3,
    ps_bufs: int = 4,
    copy_split: bool = True,
    sig_chunks: int = 3,
    in_engine: str = "gpsimd",
):
    nc = bacc.Bacc(
        "TRN2", target_bir_lowering=False, debug=False, enable_asserts=False
    )
    x = nc.dram_tensor("x", [nb, NA * NCH, GG], F32, kind="ExternalInput")
    # all constants packed in one bf16 tensor so the single const DMA has
    # >=512B per-partition runs. cols 0:92 grid8 | 92:164 inva | 164:249
    # ident (rows 0:85). g*8 values are exact in bf16 (<=600 = 7-bit
    # mantissa * 2^3); inva carries the usual 2^-9 rounding. inva stores
    # 12 repeats of (1/a_w, 1/a_h) per anchor; fix-ups read it via
    # aliased strided APs [[2,7],[2,6],[1,2]] (addresses 2j+2k+c overlap,
    # all steps nonzero -- HW-validated, unlike step-0 broadcast APs).
    IVW = 24
    CP = QCOLS  # 92 (g8 only)
    FC = NCH + NA * IVW  # 157: ident | inva (f32)
    cpk = nc.dram_tensor("cpack", [128, CP], BF16, kind="ExternalInput")
    idk = nc.dram_tensor("fconst", [128, FC], F32, kind="ExternalInput")
    out = nc.dram_tensor("out", [nb, NA, GG, NCH], BF16, kind="ExternalOutput")

    with tile.TileContext(nc) as tc, ExitStack() as ctx:
        ctx.enter_context(
            nc.allow_low_precision(
                reason="transpose-mode matmul only moves bf16 values; no accumulation"
            )
        )
        cpool = ctx.enter_context(tc.tile_pool(name="consts", bufs=1))
        inp = ctx.enter_context(tc.tile_pool(name="inp", bufs=inp_bufs))
        sp = ctx.enter_context(tc.tile_pool(name="sig", bufs=sig_bufs))
        op = ctx.enter_context(tc.tile_pool(name="outp", bufs=out_bufs))
        dp = ctx.enter_context(tc.tile_pool(name="scr", bufs=2))
        pp = ctx.enter_context(tc.tile_pool(name="ps", bufs=ps_bufs, space="PSUM"))

        cp_t = cpool.tile([128, CP], BF16)
        nc.sync.dma_start(cp_t[:], cpk[:, :])
        fc_t = cpool.tile([128, FC], F32, tag="fc")
        nc.sync.dma_start(fc_t[:], idk[:, :])
        g8_t = cp_t[:, 0:QCOLS]
        id_t = fc_t[0:NCH, 0:NCH]
        iva_t = fc_t[:, NCH : NCH + NA * IVW]

        def aliased(view, dims):
            v = view.copy()
            v.ap = type(v.ap)([list(v.ap)[0]] + dims)
            return v

        bounds = [GG * c // sig_chunks for c in range(sig_chunks + 1)]
        in_eng = getattr(nc, in_engine)
        for b in range(nb):
            for a in range(NA):
                xin = inp.tile([NCH, GG], F32, tag="xin")
                for lo, hi in zip(bounds, bounds[1:]):
                    in_eng.dma_start(
                        xin[:, lo:hi], x[b][a * NCH : (a + 1) * NCH, lo:hi]
                    )
                s = sp.tile([NCH, GG], F32, tag="s")
                for lo, hi in zip(bounds, bounds[1:]):
                    nc.scalar.activation(s[:, lo:hi], xin[:, lo:hi], Act.Sigmoid)

                o = op.tile([128, OUT_COLS], BF16, tag="o")
                w23 = dp.tile([128, QCOLS], F32, tag="w23")
                d = dp.tile([128, QCOLS], F32, tag="d")

                def fix_and_store(j0, j1, with_tail):
                    # Box fix-ups in the transposed layout for j in [j0, j1).
                    # cols 0:2 (bf16, in place): (sigmoid * 8) + grid8.
                    # cols 2:4 (f32 staging in w23 -- bf16 would cancel
                    # catastrophically in 1-s for large positive w):
                    # a*exp(w) = s*a/(1-s): d=(s-1)/a, r=1/d, out=(-s)*r,
                    # converted to bf16 on the final write into o.
                    # Split into two j-halves so each half's store can fire
                    # without waiting for the whole slab's fix-ups.
                    jn = j1 - j0
                    mv = o[:, j0 * MAIN_COLS : j1 * MAIN_COLS].rearrange(
                        "p (j kk c) -> p j kk c", j=jn, kk=KK, c=NCH
                    )
                    c01 = mv[:, :, :, 0:2]
                    c23 = mv[:, :, :, 2:4]
                    gm = g8_t[:, j0 * 12 : j1 * 12].rearrange(
                        "p (j kk c) -> p j kk c", j=jn, kk=KK, c=2
                    )
                    nc.vector.scalar_tensor_tensor(
                        c01, c01, STRIDE, gm, Alu.mult, Alu.add
                    )
                    im = aliased(
                        iva_t[:, a * IVW + 2 * j0 : (a + 1) * IVW],
                        [[2, jn], [2, KK], [1, 2]],
                    )
                    wm = w23[:, j0 * 12 : j1 * 12].rearrange(
                        "p (j kk c) -> p j kk c", j=jn, kk=KK, c=2
                    )
                    dm = d[:, j0 * 12 : j1 * 12].rearrange(
                        "p (j kk c) -> p j kk c", j=jn, kk=KK, c=2
                    )
                    nc.vector.scalar_tensor_tensor(
                        dm, wm, 1.0, im, Alu.subtract, Alu.mult
                    )
                    if with_tail:
                        tv = o[0:TI, NJ * MAIN_COLS : OUT_COLS].rearrange(
                            "p (kk c) -> p kk c", kk=TK, c=NCH
                        )
                        t01 = tv[:, :, 0:2]
                        gt = g8_t[0:TI, 84:QCOLS].rearrange(
                            "p (kk c) -> p kk c", kk=TK, c=2
                        )
                        nc.vector.scalar_tensor_tensor(
                            t01, t01, STRIDE, gt, Alu.mult, Alu.add
                        )
                        it = aliased(
                            iva_t[0:TI, a * IVW : (a + 1) * IVW], [[2, TK], [1, 2]]
                        )
                        wt = w23[0:TI, 84:QCOLS].rearrange(
                            "p (kk c) -> p kk c", kk=TK, c=2
                        )
                        dt = d[0:TI, 84:QCOLS].rearrange(
                            "p (kk c) -> p kk c", kk=TK, c=2
                        )
                        nc.vector.scalar_tensor_tensor(
                            dt, wt, 1.0, it, Alu.subtract, Alu.mult
                        )
                        nc.vector.reciprocal(
                            d[:, j0 * 12 : QCOLS], d[:, j0 * 12 : QCOLS]
                        )
                    else:
                        nc.vector.reciprocal(
                            d[:, j0 * 12 : j1 * 12], d[:, j0 * 12 : j1 * 12]
                        )
                    nc.vector.scalar_tensor_tensor(
                        c23, wm, -1.0, dm, Alu.mult, Alu.mult
                    )
                    if with_tail:
                        t23 = tv[:, :, 2:4]
                        nc.vector.scalar_tensor_tensor(
                            t23, wt, -1.0, dt, Alu.mult, Alu.mult
                        )
                    om = out[b, a][j0 * 768 : j1 * 768].rearrange(
                        "(j i kk) c -> i j kk c", j=jn, i=KI, kk=KK
                    )
                    nc.sync.dma_start(om, o[:, j0 * MAIN_COLS : j1 * MAIN_COLS])
                    if with_tail:
                        ot = out[b, a][MAIN_PX:GG].rearrange(
                            "(i kk) c -> i kk c", i=TI, kk=TK
                        )
                        nc.sync.dma_start(
                            ot, o[0:TI, NJ * MAIN_COLS : OUT_COLS]
                        )

                JS = 4
                for j in range(NJ):
                    ps = pp.tile([128, MAIN_COLS], F32, tag="ps")
                    for kk in range(KK):
                        sel = slice(j * 768 + kk, (j + 1) * 768, KK)
                        nc.tensor.transpose(
                            ps[:, kk * NCH : (kk + 1) * NCH],
                            s[:, sel],
                            id_t[0:NCH, 0:NCH],
                        )
                    dst = o[:, j * MAIN_COLS : (j + 1) * MAIN_COLS]
                    if copy_split and j % 2 == 1:
                        nc.scalar.copy(dst, ps[:])
                    else:
                        nc.vector.tensor_copy(dst, ps[:])
                    psv = ps[:].rearrange("p (kk c) -> p kk c", kk=KK, c=NCH)
                    nc.vector.tensor_copy(
                        w23[:, j * 12 : (j + 1) * 12].rearrange(
                            "p (kk c) -> p kk c", kk=KK, c=2
                        ),
                        psv[:, :, 2:4],
                    )
                    if j == JS - 1:
                        fix_and_store(0, JS, with_tail=False)
                pst = pp.tile([128, MAIN_COLS], F32, tag="ps")
                for kk in range(TK):
                    sel = slice(MAIN_PX + kk, GG, TK)
                    nc.tensor.transpose(
                        pst[0:TI, kk * NCH : (kk + 1) * NCH],
                        s[:, sel],
                        id_t[0:NCH, 0:NCH],
                    )
                nc.vector.tensor_copy(
                    o[0:TI, NJ * MAIN_COLS : OUT_COLS], pst[0:TI, 0:TAIL_COLS]
                )
                pstv = pst[0:TI, 0:TAIL_COLS].rearrange(
                    "p (kk c) -> p kk c", kk=TK, c=NCH
                )
                nc.vector.tensor_copy(
                    w23[0:TI, 84:QCOLS].rearrange("p (kk c) -> p kk c", kk=TK, c=2),
                    pstv[:, :, 2:4],
                )
                fix_and_store(JS, NJ, with_tail=True)

    nc.compile()
    return nc


def _consts(anchors: np.ndarray):
    i128 = np.arange(128)
    grid8 = np.zeros((128, QCOLS), np.float32)
    for j in range(NJ):
        for kk in range(KK):
            p = j * KI * KK + i128 * KK + kk
            grid8[:, j * 12 + kk * 2 + 0] = STRIDE * (p % G)
            grid8[:, j * 12 + kk * 2 + 1] = STRIDE * (p // G)
    for kk in range(TK):
        p = MAIN_PX + i128[:TI] * TK + kk
        grid8[:TI, 84 + kk * 2 + 0] = STRIDE * (p % G)
        grid8[:TI, 84 + kk * 2 + 1] = STRIDE * (p // G)

    IVW = 24
    inva = np.zeros((128, NA * IVW), np.float32)
    for a in range(NA):
        for m in range(IVW):
            inva[:, a * IVW + m] = 1.0 / float(anchors[a][m % 2])

    cpack = grid8.astype(ml_dtypes.bfloat16)
    fconst = np.zeros((128, NCH + NA * IVW), np.float32)
    fconst[0:NCH, 0:NCH] = np.eye(NCH, dtype=np.float32)
    fconst[:, NCH:] = inva
    return cpack, fconst


_NC_CACHE: dict[int, object] = {}

LAST_RESULTS = None


def kernel(x: np.ndarray, anchors: np.ndarray) -> np.ndarray:
    global LAST_RESULTS
    x = np.ascontiguousarray(x, dtype=np.float32)
    anchors = np.asarray(anchors, dtype=np.float32)
    B = x.shape[0]
    nb = B // N_CORES
    assert nb * N_CORES == B

    if nb not in _NC_CACHE:
        _NC_CACHE[nb] = _build(nb)
    nc = _NC_CACHE[nb]

    cpack, fconst = _consts(anchors)
    xr = x.reshape(B, NA * NCH, GG)
    in_maps = [
        {"x": xr[c * nb : (c + 1) * nb], "cpack": cpack, "fconst": fconst}
        for c in range(N_CORES)
    ]
    res = run_bass_kernel_spmd(nc, in_maps, list(range(N_CORES)))
    LAST_RESULTS = res
    outs = [
        np.asarray(res.results[c]["out"])
        .astype(np.float32)
        .reshape(nb, NA * GG, NCH)
        for c in range(N_CORES)
    ]
    return np.concatenate(outs, axis=0)


# revision 12
# speedup vs baseline: 1.0895x; 1.0895x over previous
"""Trainium2 Bass kernel for the YOLO-style DetectionLayer.

Reference computation (per batch b, anchor a, grid cell (gy, gx)):
    pred = x[b].reshape(3, 85, 76, 76)  channels-first per anchor
    bx = (sigmoid(tx) + gx) * stride        stride = 608/76 = 8
    by = (sigmoid(ty) + gy) * stride
    bw = exp(tw) * anchor_w                 (stride cancels)
    bh = exp(th) * anchor_h
    conf/cls = sigmoid(...)
    out[b, a*5776 + gy*76 + gx, :] = [bx, by, bw, bh, conf, cls0..79]

Strategy (pure data-parallel over batch, 8 cores x 4 images):
  * Per (b, a) slab: DMA [85 ch, 5776 px] f32 -> SBUF (channels on
    partitions).
  * One ACT pass: sigmoid over all 85 rows, rounding to bf16 (the
    harness tolerates rel err < 2e-2; bf16 keeps us ~3 decimal digits).
    exp is derived on DVE as s/(1-s) to avoid the ~2.7us ACT table
    switch between the sigmoid and exp sets.
  * TensorE transpose-mode matmuls (bf16, 2x fp32 rate) flip
    [85, 128px] -> PSUM [128px, 85ch].  Pixels are interleaved stride-6
    so each SBUF output partition holds 6 consecutive output rows =
    1020 contiguous bytes in DRAM per partition (>=512B keeps the DMA
    at full rate).
  * Box fix-ups run in the transposed layout where box channels are a
    few free-dim columns across all 128 partitions (3-4 DVE ops/slab).
  * One big bf16 store DMA per slab -- HALF the bytes of an f32 store.
    The host widens bf16 -> f32 at gather time (exact bit-shift).
"""

from contextlib import ExitStack

import ml_dtypes
import numpy as np

import concourse.bacc as bacc
import concourse.mybir as mybir
import concourse.tile as tile
from concourse.bass_utils import run_bass_kernel_spmd

F32 = mybir.dt.float32
BF16 = mybir.dt.bfloat16
Alu = mybir.AluOpType
Act = mybir.ActivationFunctionType

N_CORES = 8
NA = 3  # anchors
NCH = 85  # 5 + 80 classes
G = 76
GG = G * G  # 5776
STRIDE = 8.0

# pixel chunking for the transpose: 7 chunks of 128 partitions x 6 px
# (stride-6 interleave), tail chunk of 100 partitions x 4 px.
NJ, KI, KK = 7, 128, 6  # main: 7 * 768 px
TI, TK = 100, 4  # tail: 400 px
MAIN_PX = NJ * KI * KK  # 5376
MAIN_COLS = KK * NCH  # 510
TAIL_COLS = TK * NCH  # 340
OUT_COLS = NJ * MAIN_COLS + TAIL_COLS  # 3910

# grid8 / inva column layout: main j<7: q = j*12 + kk*2 + c ; tail: 84 + kk*2 + c
QCOLS = NJ * KK * 2 + TK * 2  # 92


def _build(
    nb: int,
    inp_bufs: int = 2,
    sig_bufs: int = 2,
    out_bufs: int = 3,
    ps_bufs: int = 4,
    copy_split: bool = False,
    sig_chunks: int = 3,
    in_engine: str = "gpsimd",
):
    nc = bacc.Bacc(
        "TRN2", target_bir_lowering=False, debug=False, enable_asserts=False
    )
    x = nc.dram_tensor("x", [nb, NA * NCH, GG], F32, kind="ExternalInput")
    # all constants packed in one bf16 tensor so the single const DMA has
    # >=512B per-partition runs. cols 0:92 grid8 | 92:164 inva | 164:249
    # ident (rows 0:85). g*8 values are exact in bf16 (<=600 = 7-bit
    # mantissa * 2^3); inva carries the usual 2^-9 rounding. inva stores
    # 12 repeats of (1/a_w, 1/a_h) per anchor; fix-ups read it via
    # aliased strided APs [[2,7],[2,6],[1,2]] (addresses 2j+2k+c overlap,
    # all steps nonzero -- HW-validated, unlike step-0 broadcast APs).
    IVW = 24
    CP = QCOLS  # 92 (g8 only)
    FC = NCH + NA * IVW  # 157: ident | inva (f32)
    cpk = nc.dram_tensor("cpack", [128, CP], BF16, kind="ExternalInput")
    idk = nc.dram_tensor("fconst", [128, FC], F32, kind="ExternalInput")
    out = nc.dram_tensor("out", [nb, NA, GG, NCH], BF16, kind="ExternalOutput")

    with tile.TileContext(nc) as tc, ExitStack() as ctx:
        ctx.enter_context(
            nc.allow_low_precision(
                reason="transpose-mode matmul only moves bf16 values; no accumulation"
            )
        )
        cpool = ctx.enter_context(tc.tile_pool(name="consts", bufs=1))
        inp = ctx.enter_context(tc.tile_pool(name="inp", bufs=inp_bufs))
        sp = ctx.enter_context(tc.tile_pool(name="sig", bufs=sig_bufs))
        op = ctx.enter_context(tc.tile_pool(name="outp", bufs=out_bufs))
        dp = ctx.enter_context(tc.tile_pool(name="scr", bufs=2))
        pp = ctx.enter_context(tc.tile_pool(name="ps", bufs=ps_bufs, space="PSUM"))

        cp_t = cpool.tile([128, CP], BF16)
        nc.sync.dma_start(cp_t[:], cpk[:, :])
        fc_t = cpool.tile([128, FC], F32, tag="fc")
        nc.sync.dma_start(fc_t[:], idk[:, :])
        g8_t = cp_t[:, 0:QCOLS]
        id_t = fc_t[0:NCH, 0:NCH]
        iva_t = fc_t[:, NCH : NCH + NA * IVW]

        def aliased(view, dims):
            v = view.copy()
            v.ap = type(v.ap)([list(v.ap)[0]] + dims)
            return v

        bounds = [GG * c // sig_chunks for c in range(sig_chunks + 1)]
        in_eng = getattr(nc, in_engine)
        for b in range(nb):
            for a in range(NA):
                xin = inp.tile([NCH, GG], F32, tag="xin")
                for lo, hi in zip(bounds, bounds[1:]):
                    in_eng.dma_start(
                        xin[:, lo:hi], x[b][a * NCH : (a + 1) * NCH, lo:hi]
                    )

                o = op.tile([128, OUT_COLS], BF16, tag="o")
                w23 = dp.tile([128, QCOLS], F32, tag="w23")  # raw w,h (f32)
                s23 = dp.tile([128, QCOLS], F32, tag="s23")  # sigmoid(w,h)
                d = dp.tile([128, QCOLS], F32, tag="d")

                def fix_and_store(j0, j1, with_tail):
                    # Box fix-ups in the transposed layout for j in [j0, j1).
                    # cols 0:2 (bf16, in place): (sigmoid * 8) + grid8.
                    # cols 2:4: from the RAW f32 w,h staged in w23 (bf16
                    # sigmoids would cancel catastrophically in 1-s for
                    # large positive w).  s = sigmoid(w) on ACT -- same
                    # table set as the main sigmoids, so no table reload.
                    # a*exp(w) = s*a/(1-s): d=(s-1)/a, r=1/d, out=(-s)*r,
                    # rounded to bf16 on the final write into o.
                    # Two j-halves per slab so each half's store can fire
                    # without waiting for the whole slab's fix-ups.
                    jn = j1 - j0
                    mv = o[:, j0 * MAIN_COLS : j1 * MAIN_COLS].rearrange(
                        "p (j kk c) -> p j kk c", j=jn, kk=KK, c=NCH
                    )
                    c01 = mv[:, :, :, 0:2]
                    c23 = mv[:, :, :, 2:4]
                    gm = g8_t[:, j0 * 12 : j1 * 12].rearrange(
                        "p (j kk c) -> p j kk c", j=jn, kk=KK, c=2
                    )
                    nc.vector.scalar_tensor_tensor(
                        c01, c01, STRIDE, gm, Alu.mult, Alu.add
                    )
                    nc.scalar.activation(
                        s23[:, j0 * 12 : j1 * 12],
                        w23[:, j0 * 12 : j1 * 12],
                        Act.Sigmoid,
                    )
                    im = aliased(
                        iva_t[:, a * IVW + 2 * j0 : (a + 1) * IVW],
                        [[2, jn], [2, KK], [1, 2]],
                    )
                    sm = s23[:, j0 * 12 : j1 * 12].rearrange(
                        "p (j kk c) -> p j kk c", j=jn, kk=KK, c=2
                    )
                    dm = d[:, j0 * 12 : j1 * 12].rearrange(
                        "p (j kk c) -> p j kk c", j=jn, kk=KK, c=2
                    )
                    nc.vector.scalar_tensor_tensor(
                        dm, sm, 1.0, im, Alu.subtract, Alu.mult
                    )
                    if with_tail:
                        tv = o[0:TI, NJ * MAIN_COLS : OUT_COLS].rearrange(
                            "p (kk c) -> p kk c", kk=TK, c=NCH
                        )
                        t01 = tv[:, :, 0:2]
                        gt = g8_t[0:TI, 84:QCOLS].rearrange(
                            "p (kk c) -> p kk c", kk=TK, c=2
                        )
                        nc.vector.scalar_tensor_tensor(
                            t01, t01, STRIDE, gt, Alu.mult, Alu.add
                        )
                        nc.scalar.activation(
                            s23[0:TI, 84:QCOLS], w23[0:TI, 84:QCOLS], Act.Sigmoid
                        )
                        it = aliased(
                            iva_t[0:TI, a * IVW : (a + 1) * IVW], [[2, TK], [1, 2]]
                        )
                        st = s23[0:TI, 84:QCOLS].rearrange(
                            "p (kk c) -> p kk c", kk=TK, c=2
                        )
                        dt = d[0:TI, 84:QCOLS].rearrange(
                            "p (kk c) -> p kk c", kk=TK, c=2
                        )
                        nc.vector.scalar_tensor_tensor(
                            dt, st, 1.0, it, Alu.subtract, Alu.mult
                        )
                        nc.vector.reciprocal(
                            d[:, j0 * 12 : 84], d[:, j0 * 12 : 84]
                        )
                        nc.vector.reciprocal(
                            d[0:TI, 84:QCOLS], d[0:TI, 84:QCOLS]
                        )
                    else:
                        nc.vector.reciprocal(
                            d[:, j0 * 12 : j1 * 12], d[:, j0 * 12 : j1 * 12]
                        )
                    nc.vector.scalar_tensor_tensor(
                        c23, sm, -1.0, dm, Alu.mult, Alu.mult
                    )
                    if with_tail:
                        t23 = tv[:, :, 2:4]
                        nc.vector.scalar_tensor_tensor(
                            t23, st, -1.0, dt, Alu.mult, Alu.mult
                        )
                    om = out[b, a][j0 * 768 : j1 * 768].rearrange(
                        "(j i kk) c -> i j kk c", j=jn, i=KI, kk=KK
                    )
                    nc.sync.dma_start(om, o[:, j0 * MAIN_COLS : j1 * MAIN_COLS])
                    if with_tail:
                        ot = out[b, a][MAIN_PX:GG].rearrange(
                            "(i kk) c -> i kk c", i=TI, kk=TK
                        )
                        nc.sync.dma_start(
                            ot, o[0:TI, NJ * MAIN_COLS : OUT_COLS]
                        )

                JS = 4
                for j in range(NJ):
                    ps = pp.tile([128, MAIN_COLS], F32, tag="ps")
                    for kk in range(KK):
                        sel = slice(j * 768 + kk, (j + 1) * 768, KK)
                        nc.tensor.transpose(
                            ps[:, kk * NCH : (kk + 1) * NCH],
                            xin[:, sel],
                            id_t[0:NCH, 0:NCH],
                        )
                    psv = ps[:].rearrange("p (kk c) -> p kk c", kk=KK, c=NCH)
                    nc.vector.tensor_copy(
                        w23[:, j * 12 : (j + 1) * 12].rearrange(
                            "p (kk c) -> p kk c", kk=KK, c=2
                        ),
                        psv[:, :, 2:4],
                    )
                    # sigmoid straight out of PSUM into the bf16 store tile
                    nc.scalar.activation(
                        o[:, j * MAIN_COLS : (j + 1) * MAIN_COLS],
                        ps[:],
                        Act.Sigmoid,
                    )
                    if j == JS - 1:
                        fix_and_store(0, JS, with_tail=False)
                pst = pp.tile([128, MAIN_COLS], F32, tag="ps")
                for kk in range(TK):
                    sel = slice(MAIN_PX + kk, GG, TK)
                    nc.tensor.transpose(
                        pst[0:TI, kk * NCH : (kk + 1) * NCH],
                        xin[:, sel],
                        id_t[0:NCH, 0:NCH],
                    )
                pstv = pst[0:TI, 0:TAIL_COLS].rearrange(
                    "p (kk c) -> p kk c", kk=TK, c=NCH
                )
                nc.vector.tensor_copy(
                    w23[0:TI, 84:QCOLS].rearrange("p (kk c) -> p kk c", kk=TK, c=2),
                    pstv[:, :, 2:4],
                )
                nc.scalar.activation(
                    o[0:TI, NJ * MAIN_COLS : OUT_COLS],
                    pst[0:TI, 0:TAIL_COLS],
                    Act.Sigmoid,
                )
                fix_and_store(JS, NJ, with_tail=True)

    nc.compile()
    return nc


def _consts(anchors: np.ndarray):
    i128 = np.arange(128)
    grid8 = np.zeros((128, QCOLS), np.float32)
    for j in range(NJ):
        for kk in range(KK):
            p = j * KI * KK + i128 * KK + kk
            grid8[:, j * 12 + kk * 2 + 0] = STRIDE * (p % G)
            grid8[:, j * 12 + kk * 2 + 1] = STRIDE * (p // G)
    for kk in range(TK):
        p = MAIN_PX + i128[:TI] * TK + kk
        grid8[:TI, 84 + kk * 2 + 0] = STRIDE * (p % G)
        grid8[:TI, 84 + kk * 2 + 1] = STRIDE * (p // G)

    IVW = 24
    inva = np.zeros((128, NA * IVW), np.float32)
    for a in range(NA):
        for m in range(IVW):
            inva[:, a * IVW + m] = 1.0 / float(anchors[a][m % 2])

    cpack = grid8.astype(ml_dtypes.bfloat16)
    fconst = np.zeros((128, NCH + NA * IVW), np.float32)
    fconst[0:NCH, 0:NCH] = np.eye(NCH, dtype=np.float32)
    fconst[:, NCH:] = inva
    return cpack, fconst


_NC_CACHE: dict[int, object] = {}

LAST_RESULTS = None


def kernel(x: np.ndarray, anchors: np.ndarray) -> np.ndarray:
    global LAST_RESULTS
    x = np.ascontiguousarray(x, dtype=np.float32)
    anchors = np.asarray(anchors, dtype=np.float32)
    B = x.shape[0]
    nb = B // N_CORES
    assert nb * N_CORES == B

    if nb not in _NC_CACHE:
        _NC_CACHE[nb] = _build(nb)
    nc = _NC_CACHE[nb]

    cpack, fconst = _consts(anchors)
    xr = x.reshape(B, NA * NCH, GG)
    in_maps = [
        {"x": xr[c * nb : (c + 1) * nb], "cpack": cpack, "fconst": fconst}
        for c in range(N_CORES)
    ]
    res = run_bass_kernel_spmd(nc, in_maps, list(range(N_CORES)))
    LAST_RESULTS = res
    outs = [
        np.asarray(res.results[c]["out"])
        .astype(np.float32)
        .reshape(nb, NA * GG, NCH)
        for c in range(N_CORES)
    ]
    return np.concatenate(outs, axis=0)


# revision 13
# speedup vs baseline: 1.0957x; 1.0057x over previous
"""Trainium2 Bass kernel for the YOLO-style DetectionLayer.

Reference computation (per batch b, anchor a, grid cell (gy, gx)):
    pred = x[b].reshape(3, 85, 76, 76)  channels-first per anchor
    bx = (sigmoid(tx) + gx) * stride        stride = 608/76 = 8
    by = (sigmoid(ty) + gy) * stride
    bw = exp(tw) * anchor_w                 (stride cancels)
    bh = exp(th) * anchor_h
    conf/cls = sigmoid(...)
    out[b, a*5776 + gy*76 + gx, :] = [bx, by, bw, bh, conf, cls0..79]

Strategy (pure data-parallel over batch, 8 cores x 4 images):
  * Per (b, a) slab: DMA [85 ch, 5776 px] f32 -> SBUF (channels on
    partitions).
  * One ACT pass: sigmoid over all 85 rows, rounding to bf16 (the
    harness tolerates rel err < 2e-2; bf16 keeps us ~3 decimal digits).
    exp is derived on DVE as s/(1-s) to avoid the ~2.7us ACT table
    switch between the sigmoid and exp sets.
  * TensorE transpose-mode matmuls (bf16, 2x fp32 rate) flip
    [85, 128px] -> PSUM [128px, 85ch].  Pixels are interleaved stride-6
    so each SBUF output partition holds 6 consecutive output rows =
    1020 contiguous bytes in DRAM per partition (>=512B keeps the DMA
    at full rate).
  * Box fix-ups run in the transposed layout where box channels are a
    few free-dim columns across all 128 partitions (3-4 DVE ops/slab).
  * One big bf16 store DMA per slab -- HALF the bytes of an f32 store.
    The host widens bf16 -> f32 at gather time (exact bit-shift).
"""

from contextlib import ExitStack

import ml_dtypes
import numpy as np

import concourse.bacc as bacc
import concourse.mybir as mybir
import concourse.tile as tile
from concourse.bass_utils import run_bass_kernel_spmd

F32 = mybir.dt.float32
BF16 = mybir.dt.bfloat16
Alu = mybir.AluOpType
Act = mybir.ActivationFunctionType

N_CORES = 8
NA = 3  # anchors
NCH = 85  # 5 + 80 classes
G = 76
GG = G * G  # 5776
STRIDE = 8.0

# pixel chunking for the transpose: 7 chunks of 128 partitions x 6 px
# (stride-6 interleave), tail chunk of 100 partitions x 4 px.
NJ, KI, KK = 7, 128, 6  # main: 7 * 768 px
TI, TK = 100, 4  # tail: 400 px
MAIN_PX = NJ * KI * KK  # 5376
MAIN_COLS = KK * NCH  # 510
TAIL_COLS = TK * NCH  # 340
OUT_COLS = NJ * MAIN_COLS + TAIL_COLS  # 3910

# grid8 / inva column layout: main j<7: q = j*12 + kk*2 + c ; tail: 84 + kk*2 + c
QCOLS = NJ * KK * 2 + TK * 2  # 92


def _build(
    nb: int,
    inp_bufs: int = 2,
    sig_bufs: int = 2,
    out_bufs: int = 3,
    ps_bufs: int = 4,
    copy_split: bool = False,
    sig_chunks: int = 3,
    in_engine: str = "gpsimd",
):
    nc = bacc.Bacc(
        "TRN2", target_bir_lowering=False, debug=False, enable_asserts=False
    )
    x = nc.dram_tensor("x", [nb, NA * NCH, GG], F32, kind="ExternalInput")
    # all constants packed in one bf16 tensor so the single const DMA has
    # >=512B per-partition runs. cols 0:92 grid8 | 92:164 inva | 164:249
    # ident (rows 0:85). g*8 values are exact in bf16 (<=600 = 7-bit
    # mantissa * 2^3); inva carries the usual 2^-9 rounding. inva stores
    # 12 repeats of (1/a_w, 1/a_h) per anchor; fix-ups read it via
    # aliased strided APs [[2,7],[2,6],[1,2]] (addresses 2j+2k+c overlap,
    # all steps nonzero -- HW-validated, unlike step-0 broadcast APs).
    IVW = 24
    CP = QCOLS  # 92 (g8 only)
    FC = NCH + NA * IVW  # 157: ident | inva (f32)
    cpk = nc.dram_tensor("cpack", [128, CP], BF16, kind="ExternalInput")
    idk = nc.dram_tensor("fconst", [128, FC], F32, kind="ExternalInput")
    out = nc.dram_tensor("out", [nb, NA, GG, NCH], BF16, kind="ExternalOutput")

    with tile.TileContext(nc) as tc, ExitStack() as ctx:
        ctx.enter_context(
            nc.allow_low_precision(
                reason="transpose-mode matmul only moves bf16 values; no accumulation"
            )
        )
        cpool = ctx.enter_context(tc.tile_pool(name="consts", bufs=1))
        inp = ctx.enter_context(tc.tile_pool(name="inp", bufs=inp_bufs))
        sp = ctx.enter_context(tc.tile_pool(name="sig", bufs=sig_bufs))
        op = ctx.enter_context(tc.tile_pool(name="outp", bufs=out_bufs))
        dp = ctx.enter_context(tc.tile_pool(name="scr", bufs=2))
        pp = ctx.enter_context(tc.tile_pool(name="ps", bufs=ps_bufs, space="PSUM"))

        cp_t = cpool.tile([128, CP], BF16)
        nc.sync.dma_start(cp_t[:], cpk[:, :])
        fc_t = cpool.tile([128, FC], F32, tag="fc")
        nc.sync.dma_start(fc_t[:], idk[:, :])
        g8_t = cp_t[:, 0:QCOLS]
        id_t = fc_t[0:NCH, 0:NCH]
        iva_t = fc_t[:, NCH : NCH + NA * IVW]

        def aliased(view, dims):
            v = view.copy()
            v.ap = type(v.ap)([list(v.ap)[0]] + dims)
            return v

        bounds = [GG * c // sig_chunks for c in range(sig_chunks + 1)]
        in_eng = getattr(nc, in_engine)
        for b in range(nb):
            for a in range(NA):
                xin = inp.tile([NCH, GG], F32, tag="xin")
                for lo, hi in zip(bounds, bounds[1:]):
                    in_eng.dma_start(
                        xin[:, lo:hi], x[b][a * NCH : (a + 1) * NCH, lo:hi]
                    )

                o = op.tile([128, OUT_COLS], BF16, tag="o")
                w23 = dp.tile([128, QCOLS], F32, tag="w23")  # raw w,h (f32)
                s23 = dp.tile([128, QCOLS], F32, tag="s23")  # sigmoid(-w,h)

                def fix_and_store(j0, j1, with_tail):
                    # Box fix-ups in the transposed layout for j in [j0, j1).
                    # cols 0:2 (bf16, in place): (sigmoid * 8) + grid8.
                    # cols 2:4: from the RAW f32 w,h staged in w23 (bf16
                    # sigmoids would cancel catastrophically in 1-s for
                    # large positive w).  s' = sigmoid(-w) on ACT -- same
                    # table set as the main sigmoids, so no table reload --
                    # then a*exp(w) = a/s' - a: r = 1/s', out = (r-1)*a,
                    # rounded to bf16 on the final write into o.
                    # Two j-halves per slab so each half's store can fire
                    # without waiting for the whole slab's fix-ups.
                    jn = j1 - j0
                    mv = o[:, j0 * MAIN_COLS : j1 * MAIN_COLS].rearrange(
                        "p (j kk c) -> p j kk c", j=jn, kk=KK, c=NCH
                    )
                    c01 = mv[:, :, :, 0:2]
                    c23 = mv[:, :, :, 2:4]
                    gm = g8_t[:, j0 * 12 : j1 * 12].rearrange(
                        "p (j kk c) -> p j kk c", j=jn, kk=KK, c=2
                    )
                    nc.vector.scalar_tensor_tensor(
                        c01, c01, STRIDE, gm, Alu.mult, Alu.add
                    )
                    nc.scalar.activation(
                        s23[:, j0 * 12 : j1 * 12],
                        w23[:, j0 * 12 : j1 * 12],
                        Act.Sigmoid,
                        scale=-1.0,
                    )
                    im = aliased(
                        iva_t[:, a * IVW + 2 * j0 : (a + 1) * IVW],
                        [[2, jn], [2, KK], [1, 2]],
                    )
                    sm = s23[:, j0 * 12 : j1 * 12].rearrange(
                        "p (j kk c) -> p j kk c", j=jn, kk=KK, c=2
                    )
                    if with_tail:
                        tv = o[0:TI, NJ * MAIN_COLS : OUT_COLS].rearrange(
                            "p (kk c) -> p kk c", kk=TK, c=NCH
                        )
                        t01 = tv[:, :, 0:2]
                        gt = g8_t[0:TI, 84:QCOLS].rearrange(
                            "p (kk c) -> p kk c", kk=TK, c=2
                        )
                        nc.vector.scalar_tensor_tensor(
                            t01, t01, STRIDE, gt, Alu.mult, Alu.add
                        )
                        nc.scalar.activation(
                            s23[0:TI, 84:QCOLS],
                            w23[0:TI, 84:QCOLS],
                            Act.Sigmoid,
                            scale=-1.0,
                        )
                        it = aliased(
                            iva_t[0:TI, a * IVW : (a + 1) * IVW], [[2, TK], [1, 2]]
                        )
                        st = s23[0:TI, 84:QCOLS].rearrange(
                            "p (kk c) -> p kk c", kk=TK, c=2
                        )
                        nc.vector.reciprocal(
                            s23[:, j0 * 12 : 84], s23[:, j0 * 12 : 84]
                        )
                        nc.vector.reciprocal(
                            s23[0:TI, 84:QCOLS], s23[0:TI, 84:QCOLS]
                        )
                    else:
                        nc.vector.reciprocal(
                            s23[:, j0 * 12 : j1 * 12], s23[:, j0 * 12 : j1 * 12]
                        )
                    nc.vector.scalar_tensor_tensor(
                        c23, sm, -1.0, im, Alu.add, Alu.mult
                    )
                    if with_tail:
                        t23 = tv[:, :, 2:4]
                        nc.vector.scalar_tensor_tensor(
                            t23, st, -1.0, it, Alu.add, Alu.mult
                        )
                    if with_tail:
                        ot = out[b, a][MAIN_PX:GG].rearrange(
                            "(i kk) c -> i kk c", i=TI, kk=TK
                        )
                        nc.sync.dma_start(
                            ot, o[0:TI, NJ * MAIN_COLS : OUT_COLS]
                        )
                    om = out[b, a][j0 * 768 : j1 * 768].rearrange(
                        "(j i kk) c -> i j kk c", j=jn, i=KI, kk=KK
                    )
                    nc.sync.dma_start(om, o[:, j0 * MAIN_COLS : j1 * MAIN_COLS])

                JS = 4
                for j in range(NJ):
                    ps = pp.tile([128, MAIN_COLS], F32, tag="ps")
                    for kk in range(KK):
                        sel = slice(j * 768 + kk, (j + 1) * 768, KK)
                        nc.tensor.transpose(
                            ps[:, kk * NCH : (kk + 1) * NCH],
                            xin[:, sel],
                            id_t[0:NCH, 0:NCH],
                        )
                    psv = ps[:].rearrange("p (kk c) -> p kk c", kk=KK, c=NCH)
                    nc.vector.tensor_copy(
                        w23[:, j * 12 : (j + 1) * 12].rearrange(
                            "p (kk c) -> p kk c", kk=KK, c=2
                        ),
                        psv[:, :, 2:4],
                    )
                    # sigmoid straight out of PSUM into the bf16 store tile
                    nc.scalar.activation(
                        o[:, j * MAIN_COLS : (j + 1) * MAIN_COLS],
                        ps[:],
                        Act.Sigmoid,
                    )
                    if j == JS - 1:
                        fix_and_store(0, JS, with_tail=False)
                pst = pp.tile([128, MAIN_COLS], F32, tag="ps")
                for kk in range(TK):
                    sel = slice(MAIN_PX + kk, GG, TK)
                    nc.tensor.transpose(
                        pst[0:TI, kk * NCH : (kk + 1) * NCH],
                        xin[:, sel],
                        id_t[0:NCH, 0:NCH],
                    )
                pstv = pst[0:TI, 0:TAIL_COLS].rearrange(
                    "p (kk c) -> p kk c", kk=TK, c=NCH
                )
                nc.vector.tensor_copy(
                    w23[0:TI, 84:QCOLS].rearrange("p (kk c) -> p kk c", kk=TK, c=2),
                    pstv[:, :, 2:4],
                )
                nc.scalar.activation(
                    o[0:TI, NJ * MAIN_COLS : OUT_COLS],
                    pst[0:TI, 0:TAIL_COLS],
                    Act.Sigmoid,
                )
                fix_and_store(JS, NJ, with_tail=True)

    nc.compile()
    return nc


def _consts(anchors: np.ndarray):
    i128 = np.arange(128)
    grid8 = np.zeros((128, QCOLS), np.float32)
    for j in range(NJ):
        for kk in range(KK):
            p = j * KI * KK + i128 * KK + kk
            grid8[:, j * 12 + kk * 2 + 0] = STRIDE * (p % G)
            grid8[:, j * 12 + kk * 2 + 1] = STRIDE * (p // G)
    for kk in range(TK):
        p = MAIN_PX + i128[:TI] * TK + kk
        grid8[:TI, 84 + kk * 2 + 0] = STRIDE * (p % G)
        grid8[:TI, 84 + kk * 2 + 1] = STRIDE * (p // G)

    IVW = 24
    inva = np.zeros((128, NA * IVW), np.float32)
    for a in range(NA):
        for m in range(IVW):
            inva[:, a * IVW + m] = float(anchors[a][m % 2])

    cpack = grid8.astype(ml_dtypes.bfloat16)
    fconst = np.zeros((128, NCH + NA * IVW), np.float32)
    fconst[0:NCH, 0:NCH] = np.eye(NCH, dtype=np.float32)
    fconst[:, NCH:] = inva
    return cpack, fconst


_NC_CACHE: dict[int, object] = {}

LAST_RESULTS = None


def kernel(x: np.ndarray, anchors: np.ndarray) -> np.ndarray:
    global LAST_RESULTS
    x = np.ascontiguousarray(x, dtype=np.float32)
    anchors = np.asarray(anchors, dtype=np.float32)
    B = x.shape[0]
    nb = B // N_CORES
    assert nb * N_CORES == B

    if nb not in _NC_CACHE:
        _NC_CACHE[nb] = _build(nb)
    nc = _NC_CACHE[nb]

    cpack, fconst = _consts(anchors)
    xr = x.reshape(B, NA * NCH, GG)
    in_maps = [
        {"x": xr[c * nb : (c + 1) * nb], "cpack": cpack, "fconst": fconst}
        for c in range(N_CORES)
    ]
    res = run_bass_kernel_spmd(nc, in_maps, list(range(N_CORES)))
    LAST_RESULTS = res
    outs = [
        np.asarray(res.results[c]["out"])
        .astype(np.float32)
        .reshape(nb, NA * GG, NCH)
        for c in range(N_CORES)
    ]
    return np.concatenate(outs, axis=0)
